# revision 4
# baseline (speedup 1.0000x reference)
"""Combined CE + Dice + Focal-Tversky segmentation loss on 8 Trainium2 cores.

Layout: pure data parallel, 2 images per core. Per image, class planes are
packed in "class pair" tiles [128, 4096] bf16: pair j holds class 2j on
partitions 0-63 and class 2j+1 on partitions 64-127; partition p%64 holds
pixels [(p%64)*4096, (p%64+1)*4096).

Per-pixel softmax stats are accumulated on-device into per-(image,class)
sufficient statistics (p_sum, TP, t_sum, sum of target-class logits, sum of
log-sum-exp); the final scalar combine runs on the host in float64.
"""

import sys

sys.path.insert(0, "/opt/trn_rl_repo")

import numpy as np

import concourse.bacc as bacc
import concourse.mybir as mybir
import concourse.tile as tile
from concourse.bass_utils import run_bass_kernel_spmd

B, C, H, W = 16, 6, 512, 512
NCORES = 8
BPC = B // NCORES  # images per core
HWPX = H * W  # 262144 pixels per image
PHALF = 64
FD = HWPX // PHALF  # 4096 free-dim columns per image
NPAIR = C // 2  # 3 class-pair tiles

CE_W, DICE_W, FT_W = 0.4, 0.4, 0.2
FT_ALPHA, FT_BETA, FT_GAMMA = 0.7, 0.3, 1.33

BF16 = mybir.dt.bfloat16
F32 = mybir.dt.float32
AF = mybir.ActivationFunctionType
ALU = mybir.AluOpType
NPBF16 = mybir.dt.np(BF16)

# tuning knobs
CH = 2048  # chunk free size for DVE/ACT elementwise ops
SUB = 512  # PSUM-bank sub-chunk for matmuls
IN_PLACE = True  # q overwrites E, qm overwrites M
USE_TS_ACCUM = True  # t_sum via tensor_scalar accum_out (else stats matmul)
USE_ACT_ACCUM = True  # sum(lse) via activation accum_out (else DVE reduce)


def _build(fd=FD, ch=CH, sub=SUB, bpc=BPC):
    nch = fd // ch
    nsub = ch // sub
    nc = bacc.Bacc("TRN2", target_bir_lowering=False, debug=False,
                   enable_asserts=False, num_devices=NCORES)

    lg_d = nc.dram_tensor("lg", [bpc, NPAIR, 128, fd], BF16, kind="ExternalInput")
    tg_d = nc.dram_tensor("tg", [bpc, 128, fd], BF16, kind="ExternalInput")
    wd_d = nc.dram_tensor("wd", [128, 128], BF16, kind="ExternalInput")
    ws_d = nc.dram_tensor("ws", [128, NPAIR * 4, 32], BF16, kind="ExternalInput")
    cv_d = nc.dram_tensor("cv", [128, NPAIR], F32, kind="ExternalInput")
    out_d = nc.dram_tensor("out", [128, 8 * bpc], F32, kind="ExternalOutput")

    with tile.TileContext(nc) as tc:
        with (
            tc.tile_pool(name="inp", bufs=1) as inp,
            tc.tile_pool(name="wk", bufs=2) as wk,
            tc.tile_pool(name="acc", bufs=1) as accp,
            tc.tile_pool(name="ps", bufs=3, space="PSUM") as ps,
            tc.tile_pool(name="pstat", bufs=2, space="PSUM") as pstat,
        ):
            wd_t = inp.tile([128, 128], BF16, tag="wd")
            nc.sync.dma_start(wd_t[:], wd_d.ap())
            ws_t = inp.tile([128, NPAIR * 4, 32], BF16, tag="ws")
            nc.sync.dma_start(ws_t[:], ws_d.ap())
            cv_t = inp.tile([128, NPAIR], F32, tag="cv")
            nc.sync.dma_start(cv_t[:], cv_d.ap())

            lg_t = inp.tile([128, bpc, NPAIR, fd], BF16, tag="lg")
            tg_t = inp.tile([128, bpc, fd], BF16, tag="tg")
            for b in range(bpc):
                for j in range(NPAIR):
                    nc.sync.dma_start(lg_t[:, b, j, :], lg_d.ap()[b, j])
                nc.sync.dma_start(tg_t[:, b, :], tg_d.ap()[b])

            out_sb = accp.tile([128, 8 * bpc], F32, tag="out")
            nc.vector.memset(out_sb[:], 0.0)

            for b in range(bpc):
                stats = pstat.tile([32, sub], F32, tag="stat")
                lse_acc = accp.tile([128, nch * nsub], F32, tag="lsea")
                ts_acc = accp.tile([128, NPAIR * nch], F32, tag="tsa")
                first_stat = True
                for chi in range(nch):
                    base = chi * ch
                    sl_ch = slice(base, base + ch)
                    E = []
                    for j in range(NPAIR):
                        Ej = wk.tile([128, ch], BF16, tag=f"E{j}")
                        nc.scalar.activation(Ej[:], lg_t[:, b, j, sl_ch], AF.Exp)
                        E.append(Ej)
                    R2 = wk.tile([128, ch], BF16, tag="R2")
                    for s in range(nsub):
                        ssl = slice(s * sub, (s + 1) * sub)
                        s2 = ps.tile([128, sub], F32, tag="s2")
                        for j in range(NPAIR):
                            nc.tensor.matmul(
                                s2[:], wd_t[:], E[j][:, ssl],
                                start=(j == 0), stop=(j == NPAIR - 1),
                            )
                        lse = wk.tile([128, sub], F32, tag="lse")
                        col = chi * nsub + s
                        if USE_ACT_ACCUM:
                            nc.scalar.activation(
                                lse[:], s2[:], AF.Ln,
                                accum_out=lse_acc[:, col:col + 1],
                            )
                        else:
                            nc.scalar.activation(lse[:], s2[:], AF.Ln)
                            nc.vector.tensor_reduce(
                                lse_acc[:, col:col + 1], lse[:],
                                axis=mybir.AxisListType.X, op=ALU.add,
                            )
                        nc.scalar.activation(R2[:, ssl], lse[:], AF.Exp, scale=-1.0)
                    for j in range(NPAIR):
                        Mj = wk.tile([128, ch], BF16, tag=f"M{j}")
                        tcol = j * nch + chi
                        if USE_TS_ACCUM:
                            nc.vector.tensor_scalar(
                                out=Mj[:], in0=tg_t[:, b, sl_ch],
                                scalar1=cv_t[:, j:j + 1], scalar2=0.0,
                                op0=ALU.is_equal, op1=ALU.add,
                                accum_out=ts_acc[:, tcol:tcol + 1],
                            )
                        else:
                            nc.vector.tensor_scalar(
                                out=Mj[:], in0=tg_t[:, b, sl_ch],
                                scalar1=cv_t[:, j:j + 1], scalar2=None,
                                op0=ALU.is_equal,
                            )
                        Lmj = wk.tile([128, ch], BF16, tag=f"Lm{j}")
                        nc.vector.tensor_tensor(
                            Lmj[:], lg_t[:, b, j, sl_ch], Mj[:], ALU.mult)
                        if IN_PLACE:
                            qj, qmj = E[j], Mj
                        else:
                            qj = wk.tile([128, ch], BF16, tag=f"q{j}")
                            qmj = wk.tile([128, ch], BF16, tag=f"qm{j}")
                        nc.vector.tensor_tensor(qj[:], E[j][:], R2[:], ALU.mult)
                        nc.vector.tensor_tensor(qmj[:], qj[:], Mj[:], ALU.mult)
                        quant_tiles = [(0, qj), (1, qmj), (2, Lmj)]
                        if not USE_TS_ACCUM:
                            quant_tiles.append((3, Mj))
                        for qi, qt in quant_tiles:
                            for s in range(nsub):
                                ssl = slice(s * sub, (s + 1) * sub)
                                last = (chi == nch - 1 and j == NPAIR - 1
                                        and qi == quant_tiles[-1][0]
                                        and s == nsub - 1)
                                nc.tensor.matmul(
                                    stats[:], ws_t[:, j * 4 + qi, :], qt[:, ssl],
                                    start=first_stat, stop=last)
                                first_stat = False
                # end of image: fold accumulators into output columns
                ob = 8 * b
                nc.vector.tensor_reduce(
                    out_sb[0:32, ob:ob + 1], stats[:],
                    axis=mybir.AxisListType.X, op=ALU.add)
                for j in range(NPAIR):
                    nc.vector.tensor_reduce(
                        out_sb[:, ob + 1 + j:ob + 2 + j],
                        ts_acc[:, j * nch:(j + 1) * nch],
                        axis=mybir.AxisListType.X, op=ALU.add)
                nc.vector.tensor_reduce(
                    out_sb[:, ob + 4:ob + 5], lse_acc[:],
                    axis=mybir.AxisListType.X, op=ALU.add)
            nc.sync.dma_start(out_d.ap(), out_sb[:])
    nc.compile()
    return nc


def _weights():
    k = np.arange(128)
    wd = (k[:, None] % 64 == k[None, :] % 64).astype(NPBF16)
    ws = np.zeros((128, NPAIR * 4, 32), dtype=NPBF16)
    for j in range(NPAIR):
        for qi in range(4):
            ws[:64, j * 4 + qi, 8 * qi + 2 * j] = 1
            ws[64:, j * 4 + qi, 8 * qi + 2 * j + 1] = 1
    cv = np.zeros((128, NPAIR), dtype=np.float32)
    for j in range(NPAIR):
        cv[:64, j] = 2 * j
        cv[64:, j] = 2 * j + 1
    return wd, ws, cv


def _prep_core(logits_np, targets_np, cores, bpc, fd):
    """Build per-core input maps. logits (B,C,H,W) f32, targets (B,H,W) int."""
    wd, ws, cv = _weights()
    lg = np.ascontiguousarray(logits_np.reshape(B, NPAIR, 128, fd)).astype(NPBF16)
    tghalf = targets_np.reshape(B, PHALF, fd).astype(NPBF16)
    tg = np.concatenate([tghalf, tghalf], axis=1)  # duplicate to both halves
    maps = []
    for c in range(cores):
        maps.append({
            "lg": np.ascontiguousarray(lg[c * bpc:(c + 1) * bpc]),
            "tg": np.ascontiguousarray(tg[c * bpc:(c + 1) * bpc]),
            "wd": wd, "ws": ws, "cv": cv,
        })
    return maps


def _finish(outs, bpc):
    """Host combine: outs = list of [128, 8*bpc] f32 per core."""
    p_sum = np.zeros((B, C)); tp = np.zeros((B, C))
    t_sum = np.zeros((B, C)); ceg = np.zeros(B); lse = np.zeros(B)
    for core, o in enumerate(outs):
        o = o.astype(np.float64)
        for b in range(bpc):
            img = core * bpc + b
            col = o[:, 8 * b]
            p_sum[img] = col[0:6]
            tp[img] = col[8:14]
            ceg[img] = col[16:22].sum()
            for j in range(NPAIR):
                t_sum[img, 2 * j] = o[:64, 8 * b + 1 + j].sum()
                t_sum[img, 2 * j + 1] = o[64:, 8 * b + 1 + j].sum()
            lse[img] = o[:, 8 * b + 4].sum() / 2.0
    npx = B * HWPX
    ce = (lse.sum() - ceg.sum()) / npx
    dice = (2.0 * tp + 1e-8) / (p_sum + t_sum + 1e-8)
    dice_loss = np.mean(1.0 - dice)
    fp = p_sum - tp
    fn = t_sum - tp
    tversky = (tp + 1e-6) / (tp + FT_ALPHA * fn + FT_BETA * fp + 1e-6)
    ft_loss = np.mean((1.0 - tversky) ** FT_GAMMA)
    return np.float32(CE_W * ce + DICE_W * dice_loss + FT_W * ft_loss)


_CACHED = {}


def kernel(logits, targets):
    logits = np.asarray(logits, dtype=np.float32)
    targets = np.asarray(targets)
    if "nc" not in _CACHED:
        _CACHED["nc"] = _build()
    maps = _prep_core(logits, targets, NCORES, BPC, FD)
    res = run_bass_kernel_spmd(_CACHED["nc"], maps, list(range(NCORES)))
    outs = [res.results[i]["out"] for i in range(NCORES)]
    return _finish(outs, BPC)


if __name__ == "__main__":
    rng = np.random.default_rng(0)
    logits = rng.standard_normal((B, C, H, W), dtype=np.float32)
    targets = rng.integers(0, C, size=(B, H, W)).astype(np.int64)
    got = kernel(logits, targets)

    # float64 numpy reference
    lg = logits.astype(np.float64)
    m = lg.max(axis=1, keepdims=True)
    e = np.exp(lg - m)
    s = e.sum(axis=1, keepdims=True)
    logp = lg - m - np.log(s)
    probs = e / s
    lp_t = np.take_along_axis(logp, targets[:, None], axis=1)[:, 0]
    ce = -lp_t.mean()
    oh = (targets[:, None] == np.arange(C)[None, :, None, None])
    tp = (probs * oh).sum(axis=(2, 3))
    p_sum = probs.sum(axis=(2, 3))
    t_sum = oh.sum(axis=(2, 3))
    dice = (2 * tp + 1e-8) / (p_sum + t_sum + 1e-8)
    dice_loss = np.mean(1 - dice)
    tv = (tp + 1e-6) / (tp + FT_ALPHA * (t_sum - tp) + FT_BETA * (p_sum - tp) + 1e-6)
    ft = np.mean((1 - tv) ** FT_GAMMA)
    want = CE_W * ce + DICE_W * dice_loss + FT_W * ft
    print("got", got, "want", want, "rel", abs(got - want) / abs(want))
